# revision 16
# baseline (speedup 1.0000x reference)
"""Trainium2 Bass kernel for fused sparse attention (policy-masked softmax).

Computation (per batch b):
    qkv  = x @ qkv_w.T + qkv_b                  -> q, k, v   [H heads, hd=64]
    S    = (q @ k.T) * hd**-0.5                 [H, N, N]
    P    = eps-softmax(S) with key-policy mask and eye-blend
    out  = (P @ v) @ proj_w.T + proj_b

Strategy: pure data-parallel over batch across 8 NeuronCores (4 batches
per core), fully fused on-chip per batch.  Host pre-transposes x and the
weights so the device kernel needs no transposes:
  - x^T [C, N] tiles are the shared lhsT/rhs for the QKV projections
  - q^T/k^T land as [64, N] head slices (contraction dim on partitions)
  - softmax runs in the S^T [key, query] orientation: the policy mask is a
    per-partition scalar, the attn row-sum rides along the attn@v matmul via
    an extra all-ones lhsT column (host-extended V weights with bias 1), and
    1/sum is applied via a gpsimd partition-broadcast.
  - attn output accumulates directly in proj-ready [C, N] layout.
Matmul operands are fp16 (1 cycle/row on the PE vs 2 for f32r/4 for fp32,
fp32 PSUM accumulation, ~4e-4 relative error).  Softmax skips the
max-subtraction (scores are O(1) here), which cancels exactly except for
the eps terms (~1e-8 relative).
"""

import sys

if "/opt/trn_rl_repo" not in sys.path:
    sys.path.insert(0, "/opt/trn_rl_repo")

import numpy as np

B, N, C, H = 32, 384, 768, 12
HD = C // H  # 64
NCORES = 8
BL = B // NCORES  # batches per core
EPS = 1e-6
SCALE = HD ** -0.5
P = 128
KT = C // P   # 6 contraction tiles over C
NT = N // P   # 3 tiles over sequence
VE = 65       # per-head v block in v_ext: [v(64) | ones-col]
JQK = 2 * C // P  # 12 q/k output tiles

_CACHE = {}


def _build_nc():
    import concourse.tile as tile
    from concourse import bacc, mybir

    F32 = mybir.dt.float32
    F16 = mybir.dt.float16
    EXP = mybir.ActivationFunctionType.Exp
    IDENT = mybir.ActivationFunctionType.Identity
    MULT = mybir.AluOpType.mult
    ADD = mybir.AluOpType.add
    NE = mybir.AluOpType.not_equal

    nc = bacc.Bacc(None, target_bir_lowering=False)

    xT_d = nc.declare_dram_parameter("xT", [BL, C, N], F16, isOutput=False)
    pol_d = nc.declare_dram_parameter("pol", [BL, P, NT], F32, isOutput=False)
    wqkT_d = nc.declare_dram_parameter("wqkT", [C, 2 * C], F16, isOutput=False)
    # V weights extended per head with a zero column whose bias is 1.0,
    # so the attn row-sum rides along the attn@v matmul as output row 64.
    wvT_d = nc.declare_dram_parameter("wvT", [C, H * VE], F16, isOutput=False)
    bqk_d = nc.declare_dram_parameter("bqk", [P, JQK], F32, isOutput=False)
    bv_d = nc.declare_dram_parameter("bv", [H * VE], F32, isOutput=False)
    wpT_d = nc.declare_dram_parameter("wpT", [C, C], F16, isOutput=False)
    bp_d = nc.declare_dram_parameter("bp", [C], F32, isOutput=False)
    out_d = nc.declare_dram_parameter("out", [BL, N, C], F32, isOutput=True)

    import concourse.bass as bass

    def bcast_dram(vec_ap, parts):
        # partition-broadcast a 1-D DRAM vector: step 0 over partitions
        return bass.AP(
            tensor=vec_ap.tensor,
            offset=vec_ap.offset,
            ap=[[0, parts]] + list(vec_ap.ap),
        )

    with tile.TileContext(nc) as tc:
        with (
            tc.tile_pool(name="singles", bufs=1) as singles,
            tc.tile_pool(name="xin", bufs=2) as xin,
            tc.tile_pool(name="mid", bufs=2) as mid,
            tc.tile_pool(name="eact", bufs=4) as eact,
            tc.tile_pool(name="ehatp", bufs=2) as ehatp,
            tc.tile_pool(name="small", bufs=3) as small,
            tc.tile_pool(name="outp", bufs=2) as outp,
            tc.tile_pool(name="ps3", bufs=6, space="PSUM") as ps3,
            tc.tile_pool(name="ps2", bufs=2, space="PSUM") as ps2,
        ):
            # ---- resident weights/biases
            wqk_sb = singles.tile([P, KT, 2 * C], F16)
            nc.sync.dma_start(
                out=wqk_sb, in_=wqkT_d.rearrange("(kt p) j -> p kt j", p=P)
            )
            wv_sb = singles.tile([P, KT, H * VE], F16)
            nc.sync.dma_start(
                out=wv_sb, in_=wvT_d.rearrange("(kt p) j -> p kt j", p=P)
            )
            wp_sb = singles.tile([P, KT, C], F16)
            nc.sync.dma_start(
                out=wp_sb, in_=wpT_d.rearrange("(kt p) j -> p kt j", p=P)
            )
            bqk_sb = singles.tile([P, JQK], F32)
            nc.sync.dma_start(out=bqk_sb, in_=bqk_d[:, :])
            bv_sb = singles.tile([P, H * VE], F32)
            nc.gpsimd.dma_start(out=bv_sb, in_=bcast_dram(bv_d[:], P))
            bp_sb = singles.tile([P, C], F32)
            nc.gpsimd.dma_start(out=bp_sb, in_=bcast_dram(bp_d[:], P))

            for b in range(BL):
                # ---- load x^T and policy
                xT_sb = xin.tile([P, KT, N], F16, tag="xT")
                nc.sync.dma_start(
                    out=xT_sb, in_=xT_d[b].rearrange("(kt p) n -> p kt n", p=P)
                )
                pol_sb = xin.tile([P, NT], F32, tag="pol")
                nc.sync.dma_start(out=pol_sb, in_=pol_d[b])

                # ---- blend^T tiles: blend[p, t, m] = 1 if m == t*128+p else pol[p]
                blend = xin.tile([P, NT, N], F16, tag="blend")
                for t in range(NT):
                    nc.scalar.activation(
                        out=blend[:, t, :], in_=xT_sb[:, 0, :],
                        func=IDENT, bias=pol_sb[:, t : t + 1], scale=0.0,
                    )
                    nc.gpsimd.affine_select(
                        out=blend[:, t, :], in_=blend[:, t, :],
                        compare_op=NE, fill=1.0, base=t * P,
                        pattern=[[-1, N]], channel_multiplier=1,
                    )

                # ---- q^T/k^T: [j, n] orientation, 12 j-tiles of 128
                qkT = mid.tile([P, JQK, N], F16, tag="qkT")
                for jt in range(JQK):
                    ps = ps3.tile([P, 512], F32, tag="mm")
                    for kt in range(KT):
                        nc.tensor.matmul(
                            ps[:, :N],
                            wqk_sb[:, kt, jt * P : (jt + 1) * P],
                            xT_sb[:, kt, :],
                            start=(kt == 0), stop=(kt == KT - 1),
                        )
                    # bias add + fp16 round on ACT (psum -> sbuf)
                    nc.scalar.activation(
                        out=qkT[:, jt, :], in_=ps[:, :N],
                        func=IDENT, bias=bqk_sb[:, jt : jt + 1], scale=1.0,
                    )

                # ---- v (natural [n, c] layout), ones column via wv_ext
                v_ext = mid.tile([P, NT, H * VE], F16, tag="vext")
                for nt in range(NT):
                    for c0, cw in ((0, 390), (390, 390)):
                        ps = ps3.tile([P, 512], F32, tag="mm")
                        for kt in range(KT):
                            nc.tensor.matmul(
                                ps[:, :cw],
                                xT_sb[:, kt, nt * P : (nt + 1) * P],
                                wv_sb[:, kt, c0 : c0 + cw],
                                start=(kt == 0), stop=(kt == KT - 1),
                            )
                        nc.vector.tensor_tensor(
                            out=v_ext[:, nt, c0 : c0 + cw],
                            in0=ps[:, :cw],
                            in1=bv_sb[:, c0 : c0 + cw],
                            op=ADD,
                        )

                # ---- attention per head
                oT = mid.tile([P, KT, N], F16, tag="oT")
                for h in range(H):
                    base = (h % 2) * HD
                    jq, jk = h // 2, JQK // 2 + h // 2
                    qh = qkT[base : base + HD, jq, :]
                    kh = qkT[base : base + HD, jk, :]
                    ehat = ehatp.tile([P, NT, N], F16, tag="ehat")
                    ps_ss = []
                    for mt in range(NT):
                        ps_s = ps3.tile([P, 512], F32, tag="mm")
                        nc.tensor.matmul(
                            ps_s[:, :N],
                            kh[:, mt * P : (mt + 1) * P],
                            qh,
                            start=True, stop=True,
                        )
                        ps_ss.append(ps_s)
                    eas = []
                    for mt in range(NT):
                        ea = eact.tile([P, N], F16, tag="ea")
                        nc.scalar.activation(
                            out=ea, in_=ps_ss[mt][:, :N], func=EXP, scale=SCALE,
                        )
                        eas.append(ea)
                    for mt in range(NT):
                        eng = nc.gpsimd if mt == 1 else nc.vector
                        eng.tensor_tensor(
                            out=ehat[:, mt, :], in0=eas[mt], in1=blend[:, mt, :],
                            op=MULT,
                        )
                    ps_av = ps2.tile([P, 512], F32, tag="av")
                    for mt in range(NT):
                        nc.tensor.matmul(
                            ps_av[: HD + 1, :N],
                            v_ext[:, mt, h * VE : (h + 1) * VE],
                            ehat[:, mt, :],
                            start=(mt == 0), stop=(mt == NT - 1),
                        )
                    # r = 1/rowsum (eps negligible vs rowsum >= exp(s_nn)).
                    # The sum sits at psum partition 64: evict via DVE (lane
                    # aligned), DMA-move the row to partition 0 (custom DVE
                    # ops and partition_broadcast need base partition 0).
                    su_sb = small.tile([P, N], F32, tag="su")
                    nc.vector.tensor_copy(
                        out=su_sb[HD : HD + 1, :], in_=ps_av[HD : HD + 1, :N]
                    )
                    su0_sb = small.tile([1, N], F32, tag="su0")
                    nc.sync.dma_start(out=su0_sb, in_=su_sb[HD : HD + 1, :])
                    r_sb = small.tile([1, N], F32, tag="r")
                    nc.vector.reciprocal_approx_fast(out=r_sb, in_=su0_sb)
                    rb_sb = small.tile([HD, N], F32, tag="rb")
                    nc.gpsimd.partition_broadcast(rb_sb, r_sb)
                    nc.vector.tensor_tensor(
                        out=oT[base : base + HD, jq, :],
                        in0=ps_av[:HD, :N], in1=rb_sb, op=MULT,
                    )

                # ---- output projection
                out_sb = outp.tile([P, NT, C], F32, tag="out")
                for nt in range(NT):
                    for c0, cw in ((0, 512), (512, 256)):
                        ps = ps3.tile([P, 512], F32, tag="mm")
                        for kt in range(KT):
                            nc.tensor.matmul(
                                ps[:, :cw],
                                oT[:, kt, nt * P : (nt + 1) * P],
                                wp_sb[:, kt, c0 : c0 + cw],
                                start=(kt == 0), stop=(kt == KT - 1),
                            )
                        nc.vector.tensor_add(
                            out=out_sb[:, nt, c0 : c0 + cw],
                            in0=ps[:, :cw],
                            in1=bp_sb[:, c0 : c0 + cw],
                        )
                nc.sync.dma_start(
                    out=out_d[b].rearrange("(t p) o -> p t o", p=P), in_=out_sb
                )

    nc.compile()
    return nc


def _get_nc():
    if "nc" not in _CACHE:
        _CACHE["nc"] = _build_nc()
    return _CACHE["nc"]


def kernel(x, policy, qkv_w, qkv_b, proj_w, proj_b):
    from concourse.bass_utils import run_bass_kernel_spmd

    nc = _get_nc()

    x = np.asarray(x, dtype=np.float32)
    policy = np.asarray(policy, dtype=np.float32)
    qkv_w = np.asarray(qkv_w, dtype=np.float32)
    qkv_b = np.asarray(qkv_b, dtype=np.float32)
    proj_w = np.asarray(proj_w, dtype=np.float32)
    proj_b = np.asarray(proj_b, dtype=np.float32)

    xT = np.ascontiguousarray(x.transpose(0, 2, 1)).astype(np.float16)  # [B,C,N]
    pol = np.ascontiguousarray(
        policy.reshape(B, N).reshape(B, NT, P).transpose(0, 2, 1)
    )  # [B, P, NT]
    wqkT = np.ascontiguousarray(qkv_w[: 2 * C].T).astype(np.float16)  # [C, 2C]
    wpT = np.ascontiguousarray(proj_w.T).astype(np.float16)  # [C, C]
    bqk = np.ascontiguousarray(qkv_b[: 2 * C].reshape(JQK, P).T)  # [P, 12]
    # extended V weights/bias: per-head zero column + bias 1.0 -> ones col
    idx = np.arange(C)
    vcols = (idx // HD) * VE + (idx % HD)
    wvT = np.zeros((C, H * VE), dtype=np.float16)
    wvT[:, vcols] = qkv_w[2 * C :].T.astype(np.float16)
    bv = np.zeros(H * VE, dtype=np.float32)
    bv[vcols] = qkv_b[2 * C :]
    bv[np.arange(H) * VE + HD] = 1.0

    in_maps = []
    for c in range(NCORES):
        s = slice(c * BL, (c + 1) * BL)
        in_maps.append({
            "xT": xT[s], "pol": pol[s],
            "wqkT": wqkT, "wvT": wvT, "bqk": bqk, "bv": bv,
            "wpT": wpT, "bp": proj_b,
        })

    res = run_bass_kernel_spmd(nc, in_maps, core_ids=list(range(NCORES)))
    _CACHE["last_results"] = res
    out = np.concatenate(
        [res.results[c]["out"] for c in range(NCORES)], axis=0
    ).astype(np.float32)
    return out


# revision 17
# speedup vs baseline: 2.5628x; 2.5628x over previous
"""Trainium2 Bass kernel for fused sparse attention (policy-masked softmax).

Computation (per batch b):
    qkv  = x @ qkv_w.T + qkv_b                  -> q, k, v   [H heads, hd=64]
    S    = (q @ k.T) * hd**-0.5                 [H, N, N]
    P    = eps-softmax(S) with key-policy mask and eye-blend
    out  = (P @ v) @ proj_w.T + proj_b

Strategy: pure data-parallel over batch across 8 NeuronCores (4 batches
per core), fully fused on-chip per batch.  Host pre-transposes x and the
weights so the device kernel needs no transposes:
  - x^T [C, N] tiles are the shared lhsT/rhs for the QKV projections
  - q^T/k^T land as [64, N] head slices (contraction dim on partitions)
  - softmax runs in the S^T [key, query] orientation: the policy mask is a
    per-partition scalar, the attn row-sum rides along the attn@v matmul via
    an extra all-ones lhsT column (host-extended V weights with bias 1), and
    1/sum is applied via a gpsimd partition-broadcast.
  - attn output accumulates directly in proj-ready [C, N] layout.
Matmul operands are fp16 (1 cycle/row on the PE vs 2 for f32r/4 for fp32,
fp32 PSUM accumulation, ~4e-4 relative error).  Softmax skips the
max-subtraction (scores are O(1) here), which cancels exactly except for
the eps terms (~1e-8 relative).
"""

import sys

if "/opt/trn_rl_repo" not in sys.path:
    sys.path.insert(0, "/opt/trn_rl_repo")

import numpy as np

B, N, C, H = 32, 384, 768, 12
HD = C // H  # 64
NCORES = 8
BL = B // NCORES  # batches per core
EPS = 1e-6
SCALE = HD ** -0.5
P = 128
KT = C // P   # 6 contraction tiles over C
NT = N // P   # 3 tiles over sequence
VS = 128      # per-head v stride in v_ext: [ones | 63 zeros | v(64)]
VOFF = 64     # v offset within a head's block (psum reads from base 64)
JQK = 2 * C // P  # 12 q/k output tiles

_CACHE = {}


def _build_nc():
    import concourse.tile as tile
    from concourse import bacc, mybir

    F32 = mybir.dt.float32
    F16 = mybir.dt.float16
    EXP = mybir.ActivationFunctionType.Exp
    IDENT = mybir.ActivationFunctionType.Identity
    MULT = mybir.AluOpType.mult
    ADD = mybir.AluOpType.add
    NE = mybir.AluOpType.not_equal

    nc = bacc.Bacc(None, target_bir_lowering=False)

    xT_d = nc.declare_dram_parameter("xT", [BL, C, N], F16, isOutput=False)
    pol_d = nc.declare_dram_parameter("pol", [BL, P, NT], F32, isOutput=False)
    wqkT_d = nc.declare_dram_parameter("wqkT", [C, 2 * C], F16, isOutput=False)
    # V weights extended per head to [ones-col | 63 zero-cols | v]: the
    # attn row-sum lands at psum partition 0 (legal for the custom DVE
    # recip and partition_broadcast) and attn@v at partitions 64..127.
    wvT_d = nc.declare_dram_parameter("wvT", [C, H * VS], F16, isOutput=False)
    bqk_d = nc.declare_dram_parameter("bqk", [P, JQK], F32, isOutput=False)
    bv_d = nc.declare_dram_parameter("bv", [H * VS], F32, isOutput=False)
    wpT_d = nc.declare_dram_parameter("wpT", [C, C], F16, isOutput=False)
    bp_d = nc.declare_dram_parameter("bp", [C], F32, isOutput=False)
    out_d = nc.declare_dram_parameter("out", [BL, N, C], F32, isOutput=True)

    import concourse.bass as bass

    def bcast_dram(vec_ap, parts):
        # partition-broadcast a 1-D DRAM vector: step 0 over partitions
        return bass.AP(
            tensor=vec_ap.tensor,
            offset=vec_ap.offset,
            ap=[[0, parts]] + list(vec_ap.ap),
        )

    with tile.TileContext(nc) as tc:
        with (
            tc.tile_pool(name="singles", bufs=1) as singles,
            tc.tile_pool(name="xin", bufs=2) as xin,
            tc.tile_pool(name="mid", bufs=2) as mid,
            tc.tile_pool(name="eact", bufs=4) as eact,
            tc.tile_pool(name="ehatp", bufs=2) as ehatp,
            tc.tile_pool(name="small", bufs=3) as small,
            tc.tile_pool(name="outp", bufs=2) as outp,
            tc.tile_pool(name="ps3", bufs=5, space="PSUM") as ps3,
            tc.tile_pool(name="ps2", bufs=3, space="PSUM") as ps2,
        ):
            # ---- resident weights/biases
            wqk_sb = singles.tile([P, KT, 2 * C], F16)
            nc.sync.dma_start(
                out=wqk_sb, in_=wqkT_d.rearrange("(kt p) j -> p kt j", p=P)
            )
            wv_sb = singles.tile([P, KT, H * VS], F16)
            nc.sync.dma_start(
                out=wv_sb, in_=wvT_d.rearrange("(kt p) j -> p kt j", p=P)
            )
            wp_sb = singles.tile([P, KT, C], F16)
            nc.sync.dma_start(
                out=wp_sb, in_=wpT_d.rearrange("(kt p) j -> p kt j", p=P)
            )
            bqk_sb = singles.tile([P, JQK], F32)
            nc.sync.dma_start(out=bqk_sb, in_=bqk_d[:, :])
            bv_sb = singles.tile([P, H * VS], F32)
            nc.gpsimd.dma_start(out=bv_sb, in_=bcast_dram(bv_d[:], P))
            bp_sb = singles.tile([P, C], F32)
            nc.gpsimd.dma_start(out=bp_sb, in_=bcast_dram(bp_d[:], P))

            for b in range(BL):
                # ---- load x^T and policy
                xT_sb = xin.tile([P, KT, N], F16, tag="xT")
                nc.sync.dma_start(
                    out=xT_sb, in_=xT_d[b].rearrange("(kt p) n -> p kt n", p=P)
                )
                pol_sb = xin.tile([P, NT], F32, tag="pol")
                nc.sync.dma_start(out=pol_sb, in_=pol_d[b])

                # ---- blend^T tiles: blend[p, t, m] = 1 if m == t*128+p else pol[p]
                blend = xin.tile([P, NT, N], F16, tag="blend")
                for t in range(NT):
                    nc.scalar.activation(
                        out=blend[:, t, :], in_=xT_sb[:, 0, :],
                        func=IDENT, bias=pol_sb[:, t : t + 1], scale=0.0,
                    )
                    nc.gpsimd.affine_select(
                        out=blend[:, t, :], in_=blend[:, t, :],
                        compare_op=NE, fill=1.0, base=t * P,
                        pattern=[[-1, N]], channel_multiplier=1,
                    )

                # ---- q^T/k^T: [j, n] orientation, 12 j-tiles of 128
                qkT = mid.tile([P, JQK, N], F16, tag="qkT")
                for jt in range(JQK):
                    ps = ps3.tile([P, 512], F32, tag="mm")
                    for kt in range(KT):
                        nc.tensor.matmul(
                            ps[:, :N],
                            wqk_sb[:, kt, jt * P : (jt + 1) * P],
                            xT_sb[:, kt, :],
                            start=(kt == 0), stop=(kt == KT - 1),
                        )
                    # bias add + fp16 round on ACT (psum -> sbuf)
                    nc.scalar.activation(
                        out=qkT[:, jt, :], in_=ps[:, :N],
                        func=IDENT, bias=bqk_sb[:, jt : jt + 1], scale=1.0,
                    )

                # ---- v (natural [n, c] layout), ones column via wv_ext
                v_ext = mid.tile([P, NT, H * VS], F16, tag="vext")
                for nt in range(NT):
                    for c0, cw in ((0, 512), (512, 512), (1024, 512)):
                        ps = ps3.tile([P, 512], F32, tag="mm")
                        for kt in range(KT):
                            nc.tensor.matmul(
                                ps[:, :cw],
                                xT_sb[:, kt, nt * P : (nt + 1) * P],
                                wv_sb[:, kt, c0 : c0 + cw],
                                start=(kt == 0), stop=(kt == KT - 1),
                            )
                        nc.vector.tensor_tensor(
                            out=v_ext[:, nt, c0 : c0 + cw],
                            in0=ps[:, :cw],
                            in1=bv_sb[:, c0 : c0 + cw],
                            op=ADD,
                        )

                # ---- attention per head
                oT = mid.tile([P, KT, N], F16, tag="oT")
                for h in range(H):
                    base = (h % 2) * HD
                    jq, jk = h // 2, JQK // 2 + h // 2
                    qh = qkT[base : base + HD, jq, :]
                    kh = qkT[base : base + HD, jk, :]
                    ehat = ehatp.tile([P, NT, N], F16, tag="ehat")
                    ps_ss = []
                    for mt in range(NT):
                        ps_s = ps3.tile([P, 512], F32, tag="mm")
                        nc.tensor.matmul(
                            ps_s[:, :N],
                            kh[:, mt * P : (mt + 1) * P],
                            qh,
                            start=True, stop=True,
                        )
                        ps_ss.append(ps_s)
                    eas = []
                    for mt in range(NT):
                        ea = eact.tile([P, N], F16, tag="ea")
                        nc.scalar.activation(
                            out=ea, in_=ps_ss[mt][:, :N], func=EXP, scale=SCALE,
                        )
                        eas.append(ea)
                    for mt in range(NT):
                        nc.vector.tensor_tensor(
                            out=ehat[:, mt, :], in0=eas[mt], in1=blend[:, mt, :],
                            op=MULT,
                        )
                    ps_av = ps2.tile([P, 512], F32, tag="av")
                    for mt in range(NT):
                        nc.tensor.matmul(
                            ps_av[: VOFF + HD, :N],
                            v_ext[:, mt, h * VS : (h + 1) * VS],
                            ehat[:, mt, :],
                            start=(mt == 0), stop=(mt == NT - 1),
                        )
                    # r = 1/rowsum (eps negligible vs rowsum >= exp(s_nn)).
                    # Sum row at psum partition 0; custom ops need base 0.
                    su_sb = small.tile([1, N], F32, tag="su")
                    nc.vector.tensor_copy(out=su_sb, in_=ps_av[0:1, :N])
                    r_sb = small.tile([1, N], F32, tag="r")
                    nc.vector.reciprocal_approx_fast(out=r_sb, in_=su_sb)
                    rb_sb = small.tile([HD, N], F32, tag="rb")
                    nc.gpsimd.partition_broadcast(rb_sb, r_sb)
                    nc.vector.tensor_tensor(
                        out=oT[base : base + HD, jq, :],
                        in0=ps_av[VOFF : VOFF + HD, :N], in1=rb_sb, op=MULT,
                    )

                # ---- output projection
                out_sb = outp.tile([P, NT, C], F32, tag="out")
                for nt in range(NT):
                    for c0, cw in ((0, 512), (512, 256)):
                        ps = ps3.tile([P, 512], F32, tag="mm")
                        for kt in range(KT):
                            nc.tensor.matmul(
                                ps[:, :cw],
                                oT[:, kt, nt * P : (nt + 1) * P],
                                wp_sb[:, kt, c0 : c0 + cw],
                                start=(kt == 0), stop=(kt == KT - 1),
                            )
                        nc.vector.tensor_add(
                            out=out_sb[:, nt, c0 : c0 + cw],
                            in0=ps[:, :cw],
                            in1=bp_sb[:, c0 : c0 + cw],
                        )
                nc.sync.dma_start(
                    out=out_d[b].rearrange("(t p) o -> p t o", p=P), in_=out_sb
                )

    nc.compile()
    return nc


def _get_nc():
    if "nc" not in _CACHE:
        _CACHE["nc"] = _build_nc()
    return _CACHE["nc"]


def kernel(x, policy, qkv_w, qkv_b, proj_w, proj_b):
    from concourse.bass_utils import run_bass_kernel_spmd

    nc = _get_nc()

    x = np.asarray(x, dtype=np.float32)
    policy = np.asarray(policy, dtype=np.float32)
    qkv_w = np.asarray(qkv_w, dtype=np.float32)
    qkv_b = np.asarray(qkv_b, dtype=np.float32)
    proj_w = np.asarray(proj_w, dtype=np.float32)
    proj_b = np.asarray(proj_b, dtype=np.float32)

    xT = np.ascontiguousarray(x.transpose(0, 2, 1)).astype(np.float16)  # [B,C,N]
    pol = np.ascontiguousarray(
        policy.reshape(B, N).reshape(B, NT, P).transpose(0, 2, 1)
    )  # [B, P, NT]
    wqkT = np.ascontiguousarray(qkv_w[: 2 * C].T).astype(np.float16)  # [C, 2C]
    wpT = np.ascontiguousarray(proj_w.T).astype(np.float16)  # [C, C]
    bqk = np.ascontiguousarray(qkv_b[: 2 * C].reshape(JQK, P).T)  # [P, 12]
    # extended V weights/bias: per-head zero column + bias 1.0 -> ones col
    idx = np.arange(C)
    vcols = (idx // HD) * VS + VOFF + (idx % HD)
    wvT = np.zeros((C, H * VS), dtype=np.float16)
    wvT[:, vcols] = qkv_w[2 * C :].T.astype(np.float16)
    bv = np.zeros(H * VS, dtype=np.float32)
    bv[vcols] = qkv_b[2 * C :]
    bv[np.arange(H) * VS] = 1.0

    in_maps = []
    for c in range(NCORES):
        s = slice(c * BL, (c + 1) * BL)
        in_maps.append({
            "xT": xT[s], "pol": pol[s],
            "wqkT": wqkT, "wvT": wvT, "bqk": bqk, "bv": bv,
            "wpT": wpT, "bp": proj_b,
        })

    res = run_bass_kernel_spmd(nc, in_maps, core_ids=list(range(NCORES)))
    _CACHE["last_results"] = res
    out = np.concatenate(
        [res.results[c]["out"] for c in range(NCORES)], axis=0
    ).astype(np.float32)
    return out


# revision 18
# speedup vs baseline: 2.5655x; 1.0011x over previous
"""Trainium2 Bass kernel for fused sparse attention (policy-masked softmax).

Computation (per batch b):
    qkv  = x @ qkv_w.T + qkv_b                  -> q, k, v   [H heads, hd=64]
    S    = (q @ k.T) * hd**-0.5                 [H, N, N]
    P    = eps-softmax(S) with key-policy mask and eye-blend
    out  = (P @ v) @ proj_w.T + proj_b

Strategy: pure data-parallel over batch across 8 NeuronCores (4 batches
per core), fully fused on-chip per batch.  Host pre-transposes x and the
weights so the device kernel needs no transposes:
  - x^T [C, N] tiles are the shared lhsT/rhs for the QKV projections
  - q^T/k^T land as [64, N] head slices (contraction dim on partitions)
  - softmax runs in the S^T [key, query] orientation: the policy mask is a
    per-partition scalar, the attn row-sum rides along the attn@v matmul via
    an extra all-ones lhsT column (host-extended V weights with bias 1), and
    1/sum is applied via a gpsimd partition-broadcast.
  - attn output accumulates directly in proj-ready [C, N] layout.
Matmul operands are fp16 (1 cycle/row on the PE vs 2 for f32r/4 for fp32,
fp32 PSUM accumulation, ~4e-4 relative error).  Softmax skips the
max-subtraction (scores are O(1) here), which cancels exactly except for
the eps terms (~1e-8 relative).
"""

import sys

if "/opt/trn_rl_repo" not in sys.path:
    sys.path.insert(0, "/opt/trn_rl_repo")

import numpy as np

B, N, C, H = 32, 384, 768, 12
HD = C // H  # 64
NCORES = 8
BL = B // NCORES  # batches per core
EPS = 1e-6
SCALE = HD ** -0.5
P = 128
KT = C // P   # 6 contraction tiles over C
NT = N // P   # 3 tiles over sequence
VS = 128      # per-head v stride in v_ext: [ones | 63 zeros | v(64)]
VOFF = 64     # v offset within a head's block (psum reads from base 64)
JQK = 2 * C // P  # 12 q/k output tiles

_CACHE = {}


def _build_nc():
    import concourse.tile as tile
    from concourse import bacc, mybir

    F32 = mybir.dt.float32
    F16 = mybir.dt.float16
    EXP = mybir.ActivationFunctionType.Exp
    IDENT = mybir.ActivationFunctionType.Identity
    MULT = mybir.AluOpType.mult
    ADD = mybir.AluOpType.add
    NE = mybir.AluOpType.not_equal

    nc = bacc.Bacc(None, target_bir_lowering=False)

    xT_d = nc.declare_dram_parameter("xT", [BL, C, N], F16, isOutput=False)
    pol_d = nc.declare_dram_parameter("pol", [BL, P, NT], F32, isOutput=False)
    wqkT_d = nc.declare_dram_parameter("wqkT", [C, 2 * C], F16, isOutput=False)
    # V weights extended per head to [ones-col | 63 zero-cols | v]: the
    # attn row-sum lands at psum partition 0 (legal for the custom DVE
    # recip and partition_broadcast) and attn@v at partitions 64..127.
    wvT_d = nc.declare_dram_parameter("wvT", [C, H * VS], F16, isOutput=False)
    bqk_d = nc.declare_dram_parameter("bqk", [P, JQK], F32, isOutput=False)
    bv_d = nc.declare_dram_parameter("bv", [H * VS], F32, isOutput=False)
    wpT_d = nc.declare_dram_parameter("wpT", [C, C], F16, isOutput=False)
    bp_d = nc.declare_dram_parameter("bp", [C], F32, isOutput=False)
    out_d = nc.declare_dram_parameter("out", [BL, N, C], F32, isOutput=True)

    import concourse.bass as bass

    def bcast_dram(vec_ap, parts):
        # partition-broadcast a 1-D DRAM vector: step 0 over partitions
        return bass.AP(
            tensor=vec_ap.tensor,
            offset=vec_ap.offset,
            ap=[[0, parts]] + list(vec_ap.ap),
        )

    with tile.TileContext(nc) as tc:
        with (
            tc.tile_pool(name="singles", bufs=1) as singles,
            tc.tile_pool(name="xin", bufs=BL) as xin,
            tc.tile_pool(name="mid", bufs=2) as mid,
            tc.tile_pool(name="eact", bufs=6) as eact,
            tc.tile_pool(name="ehatp", bufs=3) as ehatp,
            tc.tile_pool(name="small", bufs=3) as small,
            tc.tile_pool(name="outp", bufs=2) as outp,
            tc.tile_pool(name="ps3", bufs=6, space="PSUM") as ps3,
            tc.tile_pool(name="ps2", bufs=2, space="PSUM") as ps2,
        ):
            # ---- resident weights/biases
            wqk_sb = singles.tile([P, KT, 2 * C], F16)
            nc.sync.dma_start(
                out=wqk_sb, in_=wqkT_d.rearrange("(kt p) j -> p kt j", p=P)
            )
            wv_sb = singles.tile([P, KT, H * VS], F16)
            nc.sync.dma_start(
                out=wv_sb, in_=wvT_d.rearrange("(kt p) j -> p kt j", p=P)
            )
            wp_sb = singles.tile([P, KT, C], F16)
            nc.sync.dma_start(
                out=wp_sb, in_=wpT_d.rearrange("(kt p) j -> p kt j", p=P)
            )
            bqk_sb = singles.tile([P, JQK], F32)
            nc.sync.dma_start(out=bqk_sb, in_=bqk_d[:, :])
            bv_sb = singles.tile([P, H * VS], F32)
            nc.gpsimd.dma_start(out=bv_sb, in_=bcast_dram(bv_d[:], P))
            bp_sb = singles.tile([P, C], F32)
            nc.gpsimd.dma_start(out=bp_sb, in_=bcast_dram(bp_d[:], P))

            # ---- prefetch all batches' x^T / policy, build blend tiles
            xT_sbs, blends = [], []
            for b in range(BL):
                xT_sb = xin.tile([P, KT, N], F16, tag="xT")
                nc.sync.dma_start(
                    out=xT_sb, in_=xT_d[b].rearrange("(kt p) n -> p kt n", p=P)
                )
                pol_sb = xin.tile([P, NT], F32, tag="pol")
                nc.sync.dma_start(out=pol_sb, in_=pol_d[b])
                # blend[p, t, m] = 1 if m == t*128+p else pol[p]
                blend = xin.tile([P, NT, N], F16, tag="blend")
                for t in range(NT):
                    nc.scalar.activation(
                        out=blend[:, t, :], in_=xT_sb[:, 0, :],
                        func=IDENT, bias=pol_sb[:, t : t + 1], scale=0.0,
                    )
                    nc.gpsimd.affine_select(
                        out=blend[:, t, :], in_=blend[:, t, :],
                        compare_op=NE, fill=1.0, base=t * P,
                        pattern=[[-1, N]], channel_multiplier=1,
                    )
                xT_sbs.append(xT_sb)
                blends.append(blend)

            for b in range(BL):
                xT_sb, blend = xT_sbs[b], blends[b]

                # ---- q^T/k^T: [j, n] orientation, 12 j-tiles of 128
                qkT = mid.tile([P, JQK, N], F16, tag="qkT")
                for jt in range(JQK):
                    ps = ps3.tile([P, 512], F32, tag="mm")
                    for kt in range(KT):
                        nc.tensor.matmul(
                            ps[:, :N],
                            wqk_sb[:, kt, jt * P : (jt + 1) * P],
                            xT_sb[:, kt, :],
                            start=(kt == 0), stop=(kt == KT - 1),
                        )
                    # bias add + fp16 round on ACT (psum -> sbuf)
                    nc.scalar.activation(
                        out=qkT[:, jt, :], in_=ps[:, :N],
                        func=IDENT, bias=bqk_sb[:, jt : jt + 1], scale=1.0,
                    )

                # ---- v (natural [n, c] layout), ones column via wv_ext
                v_ext = mid.tile([P, NT, H * VS], F16, tag="vext")
                for nt in range(NT):
                    for c0, cw in ((0, 512), (512, 512), (1024, 512)):
                        ps = ps3.tile([P, 512], F32, tag="mm")
                        for kt in range(KT):
                            nc.tensor.matmul(
                                ps[:, :cw],
                                xT_sb[:, kt, nt * P : (nt + 1) * P],
                                wv_sb[:, kt, c0 : c0 + cw],
                                start=(kt == 0), stop=(kt == KT - 1),
                            )
                        nc.vector.tensor_tensor(
                            out=v_ext[:, nt, c0 : c0 + cw],
                            in0=ps[:, :cw],
                            in1=bv_sb[:, c0 : c0 + cw],
                            op=ADD,
                        )

                # ---- attention, software-pipelined over heads: the S
                # matmuls of head h+1 are issued before the attn@v of head
                # h so the PE never waits on the exp/mask chain.
                oT = mid.tile([P, KT, N], F16, tag="oT")

                def s_phase(h):
                    base = (h % 2) * HD
                    jq, jk = h // 2, JQK // 2 + h // 2
                    qh = qkT[base : base + HD, jq, :]
                    kh = qkT[base : base + HD, jk, :]
                    ps_ss = []
                    for mt in range(NT):
                        ps_s = ps3.tile([P, 512], F32, tag="mm")
                        nc.tensor.matmul(
                            ps_s[:, :N],
                            kh[:, mt * P : (mt + 1) * P],
                            qh,
                            start=True, stop=True,
                        )
                        ps_ss.append(ps_s)
                    return ps_ss

                def softmax_av_phase(h, ps_ss):
                    base = (h % 2) * HD
                    jq = h // 2
                    ehat = ehatp.tile([P, NT, N], F16, tag="ehat")
                    eas = []
                    for mt in range(NT):
                        ea = eact.tile([P, N], F16, tag="ea")
                        nc.scalar.activation(
                            out=ea, in_=ps_ss[mt][:, :N], func=EXP, scale=SCALE,
                        )
                        eas.append(ea)
                    for mt in range(NT):
                        nc.vector.tensor_tensor(
                            out=ehat[:, mt, :], in0=eas[mt], in1=blend[:, mt, :],
                            op=MULT,
                        )
                    ps_av = ps2.tile([P, 512], F32, tag="av")
                    for mt in range(NT):
                        nc.tensor.matmul(
                            ps_av[: VOFF + HD, :N],
                            v_ext[:, mt, h * VS : (h + 1) * VS],
                            ehat[:, mt, :],
                            start=(mt == 0), stop=(mt == NT - 1),
                        )
                    # r = 1/rowsum (eps negligible vs rowsum >= exp(s_nn)).
                    # Sum row at psum partition 0; custom ops need base 0.
                    su_sb = small.tile([1, N], F32, tag="su")
                    nc.vector.tensor_copy(out=su_sb, in_=ps_av[0:1, :N])
                    r_sb = small.tile([1, N], F32, tag="r")
                    nc.vector.reciprocal_approx_fast(out=r_sb, in_=su_sb)
                    rb_sb = small.tile([HD, N], F32, tag="rb")
                    nc.gpsimd.partition_broadcast(rb_sb, r_sb)
                    nc.vector.tensor_tensor(
                        out=oT[base : base + HD, jq, :],
                        in0=ps_av[VOFF : VOFF + HD, :N], in1=rb_sb, op=MULT,
                    )

                pending = s_phase(0)
                for h in range(H):
                    nxt = s_phase(h + 1) if h + 1 < H else None
                    softmax_av_phase(h, pending)
                    pending = nxt

                # ---- output projection
                out_sb = outp.tile([P, NT, C], F32, tag="out")
                for nt in range(NT):
                    for c0, cw in ((0, 512), (512, 256)):
                        ps = ps3.tile([P, 512], F32, tag="mm")
                        for kt in range(KT):
                            nc.tensor.matmul(
                                ps[:, :cw],
                                oT[:, kt, nt * P : (nt + 1) * P],
                                wp_sb[:, kt, c0 : c0 + cw],
                                start=(kt == 0), stop=(kt == KT - 1),
                            )
                        nc.vector.tensor_add(
                            out=out_sb[:, nt, c0 : c0 + cw],
                            in0=ps[:, :cw],
                            in1=bp_sb[:, c0 : c0 + cw],
                        )
                nc.sync.dma_start(
                    out=out_d[b].rearrange("(t p) o -> p t o", p=P), in_=out_sb
                )

    nc.compile()
    return nc


def _get_nc():
    if "nc" not in _CACHE:
        _CACHE["nc"] = _build_nc()
    return _CACHE["nc"]


def kernel(x, policy, qkv_w, qkv_b, proj_w, proj_b):
    from concourse.bass_utils import run_bass_kernel_spmd

    nc = _get_nc()

    x = np.asarray(x, dtype=np.float32)
    policy = np.asarray(policy, dtype=np.float32)
    qkv_w = np.asarray(qkv_w, dtype=np.float32)
    qkv_b = np.asarray(qkv_b, dtype=np.float32)
    proj_w = np.asarray(proj_w, dtype=np.float32)
    proj_b = np.asarray(proj_b, dtype=np.float32)

    xT = np.ascontiguousarray(x.transpose(0, 2, 1)).astype(np.float16)  # [B,C,N]
    pol = np.ascontiguousarray(
        policy.reshape(B, N).reshape(B, NT, P).transpose(0, 2, 1)
    )  # [B, P, NT]
    wqkT = np.ascontiguousarray(qkv_w[: 2 * C].T).astype(np.float16)  # [C, 2C]
    wpT = np.ascontiguousarray(proj_w.T).astype(np.float16)  # [C, C]
    bqk = np.ascontiguousarray(qkv_b[: 2 * C].reshape(JQK, P).T)  # [P, 12]
    # extended V weights/bias: per-head zero column + bias 1.0 -> ones col
    idx = np.arange(C)
    vcols = (idx // HD) * VS + VOFF + (idx % HD)
    wvT = np.zeros((C, H * VS), dtype=np.float16)
    wvT[:, vcols] = qkv_w[2 * C :].T.astype(np.float16)
    bv = np.zeros(H * VS, dtype=np.float32)
    bv[vcols] = qkv_b[2 * C :]
    bv[np.arange(H) * VS] = 1.0

    in_maps = []
    for c in range(NCORES):
        s = slice(c * BL, (c + 1) * BL)
        in_maps.append({
            "xT": xT[s], "pol": pol[s],
            "wqkT": wqkT, "wvT": wvT, "bqk": bqk, "bv": bv,
            "wpT": wpT, "bp": proj_b,
        })

    res = run_bass_kernel_spmd(nc, in_maps, core_ids=list(range(NCORES)))
    _CACHE["last_results"] = res
    out = np.concatenate(
        [res.results[c]["out"] for c in range(NCORES)], axis=0
    ).astype(np.float32)
    return out
